# revision 1
# baseline (speedup 1.0000x reference)
# kernel_pair.py — pair-merged variant: GPSIMD ops span 2 mega-tiles.
# Race-oracle probing showed GPSIMD per-op overhead ~1.4 us (its streaming
# rate is near line-rate), so its 16 ops are merged into 8 double-size ops
# over image-pairs, and the row split rebalanced: GPSIMD takes out rows
# 0..29 (top), DVE rows 30..61.
import numpy as np

N_CORES = 8
N, C = 16, 256
H = W = 64
OH = OW = 62
P = 128
IMGS_PER_CORE = (N * C) // N_CORES    # 512
NPAIR = 2                             # pairs of 128-image tiles

G_OUT = 32                            # GPSIMD out rows 0..31
G_MID = G_OUT + 2                     # G mid rows 0..G_OUT+1
D_OUT = OH - G_OUT                    # DVE out rows G_OUT..61
D_MID = H - G_OUT                     # DVE mid rows G_OUT..63

_nc_cache = {}


def _split_multiwait(nc, max_waits=1):
    import concourse.mybir as mb

    for f in nc.m.functions:
        for b in f.blocks:
            new_list = []
            for inst in b.instructions:
                si = getattr(inst, "sync_info", None)
                if si is not None and len(si.on_wait) > max_waits:
                    waits = list(si.on_wait)
                    extra, keep = waits[:-max_waits], waits[-max_waits:]
                    for k, w in enumerate(extra):
                        es = mb.InstEventSemaphore(
                            name=f"{inst.name}-esw{k}", ins=[], outs=[],
                            engine=inst.engine)
                        es.sync_info = mb.SyncInfo(on_wait=[w], on_update=[])
                        nc.register_instruction(es)
                        new_list.append(es)
                    inst.sync_info = mb.SyncInfo(
                        on_wait=keep, on_update=list(si.on_update))
                new_list.append(inst)
            b.instructions[:] = new_list


def _build_nc():
    import concourse.bass as bass
    import concourse.mybir as mybir
    from concourse.tile import TileContext

    f32 = mybir.dt.float32
    g, gm, d, dm = G_OUT, G_MID, D_OUT, D_MID

    nc = bass.Bass()
    x = nc.declare_dram_parameter("x", [IMGS_PER_CORE, H, W], f32, isOutput=False)
    o = nc.declare_dram_parameter("o", [IMGS_PER_CORE, OH, OW], f32, isOutput=True)

    with TileContext(nc) as tc:
        with (
            tc.tile_pool(name="xp", bufs=NPAIR) as xp,
            tc.tile_pool(name="mgp", bufs=NPAIR) as mgp,
            tc.tile_pool(name="mdp", bufs=NPAIR) as mdp,
            tc.tile_pool(name="op", bufs=NPAIR) as op,
        ):
            for p in range(NPAIR):
                i0 = p * 2 * P        # first image of the pair
                xb = xp.tile([P, 2, H, W], f32)
                # GPSIMD's rows (top) first so the merged op starts early,
                # then DVE's rows per member.
                nc.sync.dma_start(out=xb[:, 0, 0:gm, :], in_=x[i0:i0 + P, 0:gm])
                nc.sync.dma_start(out=xb[:, 1, 0:gm, :], in_=x[i0 + P:i0 + 2 * P, 0:gm])
                nc.sync.dma_start(out=xb[:, 0, gm:H, :], in_=x[i0:i0 + P, gm:H])
                nc.sync.dma_start(out=xb[:, 1, gm:H, :], in_=x[i0 + P:i0 + 2 * P, gm:H])

                ot = op.tile([P, 2, OH, OW], f32)

                # ---- GPSIMD: pair-merged 4-op chain, out rows 0..g-1 ----
                mg = mgp.tile([P, 2, gm, OW], f32)
                nc.gpsimd.tensor_add(
                    out=mg[:], in0=xb[:, :, 0:gm, 0:62], in1=xb[:, :, 0:gm, 1:63])
                nc.gpsimd.tensor_add(
                    out=mg[:], in0=mg[:], in1=xb[:, :, 0:gm, 2:64])
                nc.gpsimd.tensor_add(
                    out=ot[:, :, 0:g, :], in0=mg[:, :, 0:g, :], in1=mg[:, :, 1:g + 1, :])
                nc.gpsimd.tensor_add(
                    out=ot[:, :, 0:g, :], in0=ot[:, :, 0:g, :], in1=mg[:, :, 2:g + 2, :])

                # ---- DVE: per-member chains, out rows g..61; per-member
                # 1/9 on ScalarE right after each chain so stores fire early
                nc.scalar.mul(out=ot[:, :, 0:g, :], in_=ot[:, :, 0:g, :], mul=1.0 / 9.0)
                last = (p == NPAIR - 1)
                for m in range(2):
                    md = mdp.tile([P, dm, OW], f32)
                    nc.vector.tensor_add(
                        out=md[:], in0=xb[:, m, g:H, 0:62], in1=xb[:, m, g:H, 1:63])
                    nc.vector.tensor_add(
                        out=md[:], in0=md[:], in1=xb[:, m, g:H, 2:64])
                    nc.vector.tensor_add(
                        out=ot[:, m, g:OH, :], in0=md[:, 0:d, :], in1=md[:, 1:d + 1, :])
                    if last and m == 1:
                        # final chain: split V2/scale at row RS so the last
                        # store is half-sized (shorter pipeline tail)
                        rs = 46
                        nc.vector.tensor_add(
                            out=ot[:, 1, g:rs, :], in0=ot[:, 1, g:rs, :],
                            in1=md[:, 2:rs - g + 2, :])
                        nc.scalar.mul(out=ot[:, 1, g:rs, :],
                                      in_=ot[:, 1, g:rs, :], mul=1.0 / 9.0)
                        nc.vector.tensor_add(
                            out=ot[:, 1, rs:OH, :], in0=ot[:, 1, rs:OH, :],
                            in1=md[:, rs - g + 2:d + 2, :])
                        nc.scalar.mul(out=ot[:, 1, rs:OH, :],
                                      in_=ot[:, 1, rs:OH, :], mul=1.0 / 9.0)
                    else:
                        nc.vector.tensor_add(
                            out=ot[:, m, g:OH, :], in0=ot[:, m, g:OH, :],
                            in1=md[:, 2:d + 2, :])
                        nc.scalar.mul(out=ot[:, m, g:OH, :],
                                      in_=ot[:, m, g:OH, :], mul=1.0 / 9.0)

                # stores per member (contiguous runs per partition)
                nc.sync.dma_start(out=o[i0:i0 + P], in_=ot[:, 0, :, :])
                if last:
                    # final two stores on different HWDGE rings: they drain
                    # in parallel instead of serializing on one ring
                    rs = 46
                    nc.sync.dma_start(
                        out=o[i0 + P:i0 + 2 * P, 0:rs, :], in_=ot[:, 1, 0:rs, :])
                    nc.scalar.dma_start(
                        out=o[i0 + P:i0 + 2 * P, rs:OH, :], in_=ot[:, 1, rs:OH, :])
                else:
                    nc.sync.dma_start(out=o[i0 + P:i0 + 2 * P], in_=ot[:, 1, :, :])

    _split_multiwait(nc)
    nc.finalize()
    return nc


def _get_nc():
    if "nc" not in _nc_cache:
        _nc_cache["nc"] = _build_nc()
    return _nc_cache["nc"]


def run(x, trace=False, **spmd_kwargs):
    from concourse.bass_utils import run_bass_kernel_spmd

    x = np.ascontiguousarray(np.asarray(x, dtype=np.float32))
    assert x.shape == (N, C, H, W), x.shape
    shards = x.reshape(N_CORES, IMGS_PER_CORE, H, W)
    in_maps = [{"x": shards[c]} for c in range(N_CORES)]
    nc = _get_nc()
    res = run_bass_kernel_spmd(
        nc, in_maps, list(range(N_CORES)), trace=trace, **spmd_kwargs
    )
    out = np.stack([res.results[c]["o"] for c in range(N_CORES)], axis=0)
    return out.reshape(N, C, OH, OW), res


def kernel(x):
    out, _ = run(x, trace=False)
    return out



# revision 3
# speedup vs baseline: 1.3805x; 1.3805x over previous
# 3x3/stride-1 VALID avg-pool over (16,256,64,64) f32, data-parallel over
# 8 NeuronCores (512 images/core, one image per SBUF partition), all four
# engine queues balanced (~26-29us each in the v1 cost model vs 47us of
# DMA on the baseline's single SP queue):
#   SP   : top loads (rows [0,RT)) + most stores
#   Act  : leading small chunk (its HWDGE queue issues first ~0.8us),
#          bottom loads split at PM, and all 1/9 scales (in-place
#          activation mul, per row-chunk so stores fire early)
#   Pool : horizontal 3-sums (mid) rows [0,PM), contiguous 2-add chains
#          in row chunks (GPSIMD-safe APs), plus two relief stores
#   DVE  : mid rows [PM,64) + all vertical 3-sums via an even/odd stride
#          trick (1.5 adds per element)
# Block 0 is chunked finely to prime the pipeline, block 3 to shorten the
# tail; final stores fan out across SP/Pool queues.
# (tensor_tensor_reduce would fuse the scale for free but does not survive
# neuronxcc codegen -- "ISA wrong length" -- so scales live on Act.)
import numpy as np

MID_BF16 = False

N_CORES = 8
N, C = 16, 256
H = W = 64
OH = OW = 62
P = 128
IMGS_PER_CORE = (N * C) // N_CORES    # 512
NBLK = 4
RT = 34
PM = 58

_nc_cache = {}


def _split_multiwait(nc, max_waits=1):
    import concourse.mybir as mb

    for f in nc.m.functions:
        for b in f.blocks:
            new_list = []
            for inst in b.instructions:
                si = getattr(inst, "sync_info", None)
                if si is not None and len(si.on_wait) > max_waits:
                    waits = list(si.on_wait)
                    extra, keep = waits[:-max_waits], waits[-max_waits:]
                    for k, w in enumerate(extra):
                        es = mb.InstEventSemaphore(
                            name=f"{inst.name}-esw{k}", ins=[], outs=[],
                            engine=inst.engine)
                        es.sync_info = mb.SyncInfo(on_wait=[w], on_update=[])
                        nc.register_instruction(es)
                        new_list.append(es)
                    inst.sync_info = mb.SyncInfo(
                        on_wait=keep, on_update=list(si.on_update))
                new_list.append(inst)
            b.instructions[:] = new_list


def _build_nc():
    import concourse.bass as bass
    import concourse.mybir as mybir
    from concourse.tile import TileContext

    f32 = mybir.dt.float32
    fmid = mybir.dt.bfloat16 if MID_BF16 else f32
    add = mybir.AluOpType.add
    mx = mybir.AluOpType.max

    nc = bass.Bass()
    x = nc.declare_dram_parameter("x", [IMGS_PER_CORE, H, W], f32, isOutput=False)
    o = nc.declare_dram_parameter("o", [IMGS_PER_CORE, OH, OW], f32, isOutput=True)

    with TileContext(nc) as tc:
        with (
            tc.tile_pool(name="xp", bufs=4) as xp,
            tc.tile_pool(name="mgp", bufs=3) as mgp,
            tc.tile_pool(name="ttp", bufs=2) as ttp,
            tc.tile_pool(name="tvp", bufs=3) as tvp,
            tc.tile_pool(name="op", bufs=3) as op,
            tc.tile_pool(name="wp", bufs=1) as wp,
        ):
            xbs = [xp.tile([P, H, W], f32, name="xb") for _ in range(NBLK)]

            def ld(eng, b, r0, r1):
                eng.dma_start(out=xbs[b][:, r0:r1, :],
                              in_=x[b * P:(b + 1) * P, r0:r1])

            # ---- loads ----
            # Act first: tiny top chunk of b0 (Act's queue issues earliest),
            # then b0's bottom tail (DVE trick-mid input), then the rest.
            ld(nc.scalar, 0, 0, 17)
            ld(nc.scalar, 0, PM, H)
            ld(nc.scalar, 0, RT, PM)
            ld(nc.sync, 0, 17, RT)
            for b in range(1, NBLK):
                ld(nc.sync, b, 0, RT)
            for b in range(1, NBLK):
                ld(nc.scalar, b, PM, H)
                ld(nc.scalar, b, RT, PM)

            acc = wp.tile([P, 32], f32)
            nacc = [0]

            def ttr(out, in0, in1):
                a = acc[:, nacc[0] % 32:nacc[0] % 32 + 1]
                nacc[0] += 1
                nc.vector.tensor_tensor_reduce(
                    out=out, in0=in0, in1=in1, scale=1.0 / 9.0, scalar=0.0,
                    op0=add, op1=mx, accum_out=a, opt_aps=False)

            mgs, ots = {}, {}

            def get_mg(b):
                if b not in mgs:
                    mgs[b] = mgp.tile([P, H, OW], fmid, name="mg")
                return mgs[b]

            def get_ot(b):
                if b not in ots:
                    ots[b] = op.tile([P, OH, OW], f32, name="ot")
                return ots[b]

            def pool_mid(b, r0, r1):
                xb, mg = xbs[b], get_mg(b)
                nc.gpsimd.tensor_add(
                    out=mg[:, r0:r1, :], in0=xb[:, r0:r1, 0:62],
                    in1=xb[:, r0:r1, 1:63])
                nc.gpsimd.tensor_add(
                    out=mg[:, r0:r1, :], in0=mg[:, r0:r1, :],
                    in1=xb[:, r0:r1, 2:64])

            def dve_trick_mid(b):
                # mid rows [PM,64) from x rows [PM,64)
                xb, mg = xbs[b], get_mg(b)
                nr = H - PM
                tt = ttp.tile([P, nr, 31], fmid, name="tt")
                nc.vector.tensor_add(
                    out=tt[:], in0=xb[:, PM:H, 1:62:2], in1=xb[:, PM:H, 2:63:2])
                nc.vector.tensor_add(
                    out=mg[:, PM:H, 0:61:2], in0=xb[:, PM:H, 0:61:2],
                    in1=tt[:, 0:nr, :])
                nc.vector.tensor_add(
                    out=mg[:, PM:H, 1:62:2], in0=tt[:, 0:nr, :],
                    in1=xb[:, PM:H, 3:64:2])

            def dve_vert(b, r0, r1):
                # out rows [r0,r1), r0/r1 even: needs mg rows r0..r1+1
                mg, ot = get_mg(b), get_ot(b)
                nh = (r1 - r0) // 2
                tv = tvp.tile([P, nh, OW], fmid, name="tv")
                nc.vector.tensor_add(
                    out=tv[:], in0=mg[:, r0 + 1:r1:2, :], in1=mg[:, r0 + 2:r1 + 1:2, :])
                nc.vector.tensor_add(
                    out=ot[:, r0:r1 - 1:2, :], in0=mg[:, r0:r1 - 1:2, :],
                    in1=tv[:, 0:nh, :])
                nc.vector.tensor_add(
                    out=ot[:, r0 + 1:r1:2, :], in0=tv[:, 0:nh, :],
                    in1=mg[:, r0 + 3:r1 + 2:2, :])

            def scale(b, r0, r1):
                ot = get_ot(b)
                nc.scalar.mul(out=ot[:, r0:r1, :], in_=ot[:, r0:r1, :],
                              mul=1.0 / 9.0)

            def store(b, r0, r1, eng):
                i0 = b * P
                ot = get_ot(b)
                eng.dma_start(out=o[i0:i0 + P, r0:r1, :], in_=ot[:, r0:r1, :])

            # ---- Pool chunk schedule ----
            # b0 finely chunked for the ramp; b3 split so the tail chain is
            # short; middles split A/B to keep DVE fed with bounded lag.
            pool_chunks = {
                0: [(0, 16), (16, RT), (RT, PM)],
                1: [(0, RT), (RT, PM)],
                2: [(0, RT), (RT, PM)],
                3: [(0, 32), (32, PM)],
            }
            # DVE per-block vert chunks (b0 split for the ramp)
            vert_chunks = {
                0: [(0, 14), (14, 30), (30, OH)],
                1: [(0, 30), (30, OH)],
                2: [(0, 30), (30, OH)],
                3: [(0, 30), (30, 46), (46, 54), (54, OH)],
            }

            # interleaved emission (per-queue program order):
            # store queue per (block, half): Pool relieves SP/Act midway
            st_q = {
                (0, 0): nc.sync, (0, 1): nc.sync,
                (1, 0): nc.sync, (1, 1): nc.gpsimd,
                (2, 0): nc.sync, (2, 1): nc.gpsimd,
                (3, 0): nc.sync, (3, 1): nc.sync,
            }
            # warm the Copy activation table during the load phase
            warm = wp.tile([P, 1], f32, name="warm")
            nc.vector.memset(warm[:], 0.0)
            nc.scalar.mul(out=warm[:], in_=warm[:], mul=1.0)

            dve_trick_mid(0)
            for b in range(NBLK):
                for (r0, r1) in pool_chunks[b]:
                    pool_mid(b, r0, r1)
                if b + 1 < NBLK:
                    dve_trick_mid(b + 1)
                for (r0, r1) in vert_chunks[b]:
                    dve_vert(b, r0, r1)
                    scale(b, r0, r1)
                if b < NBLK - 1:
                    store(b, 0, 30, st_q[(b, 0)])
                    store(b, 30, OH, st_q[(b, 1)])
                else:
                    store(b, 0, 15, nc.sync)
                    store(b, 15, 30, nc.sync)
                    store(b, 30, 46, nc.gpsimd)
                    store(b, 46, 54, nc.sync)
                    store(b, 54, OH, nc.sync)

    _split_multiwait(nc)
    nc.finalize()
    return nc


def _get_nc():
    if "nc" not in _nc_cache:
        _nc_cache["nc"] = _build_nc()
    return _nc_cache["nc"]


def run(x, trace=False, **spmd_kwargs):
    from concourse.bass_utils import run_bass_kernel_spmd

    x = np.ascontiguousarray(np.asarray(x, dtype=np.float32))
    assert x.shape == (N, C, H, W), x.shape
    shards = x.reshape(N_CORES, IMGS_PER_CORE, H, W)
    in_maps = [{"x": shards[c]} for c in range(N_CORES)]
    nc = _get_nc()
    res = run_bass_kernel_spmd(
        nc, in_maps, list(range(N_CORES)), trace=trace, **spmd_kwargs
    )
    out = np.stack([res.results[c]["o"] for c in range(N_CORES)], axis=0)
    return out.reshape(N, C, OH, OW), res


def kernel(x):
    out, _ = run(x, trace=False)
    return out
